# revision 23
# baseline (speedup 1.0000x reference)
"""Multi-head causal self-attention (D=768, H=12, S=4096) on 8 Trainium2 cores.

Sharding: 4 head-groups (3 heads each) x 2 interleaved query-sets.
Core c = 2*g + s owns head-group g (heads 3g..3g+2) and query 128-row
blocks s, s+2, s+4, ... (even/odd interleave balances the causal
triangle).  Every core runs the SAME program; per-core behaviour is
driven entirely by input data (weight slices, gathered query columns,
additive causal masks).  Each core produces a partial [2048, 768]
output (its heads pushed through its slice of Wo, all biases folded
in); the host sums the 4 group partials per query-set and re-interleaves
rows.

Layout notes (per core):
  - scores are computed transposed: S_T[k, q] = K_h Q_h^T so the PV
    matmul needs no transposes; the softmax denominator comes from a
    ones-column appended to V.
  - softmax skips max-subtraction (scores are O(1) by construction).
  - causal masking is additive mask data (0 / -30000) applied to score
    PSUM with DVE before the exp, covering only the boundary k-blocks.
"""

import numpy as np

D = 768
S = 4096
H = 12
HD = 64
NG = 4          # head groups
GH = 3          # heads per group
GD = GH * HD    # 192 dims per group
SL = S // 2     # local queries per core (2048)
P = 128
NC = D // P     # 6 contraction chunks
QG = 4          # query groups per core (512 q each)
QGS = 512
NKB = S // P    # 32 key blocks
NQB = SL // P   # 16 local query tiles
MASKVAL = -30000.0

_CACHE = {}


def _build_program():
    import concourse.bacc as bacc
    import concourse.mybir as mybir
    import concourse.tile as tile
    from contextlib import ExitStack

    bf16 = mybir.dt.bfloat16
    f32 = mybir.dt.float32
    f32r = mybir.dt.float32r

    nc = bacc.Bacc("TRN2", target_bir_lowering=False, debug=False, num_devices=8)

    xt = nc.dram_tensor("xt", [D, S], bf16, kind="ExternalInput").ap()
    xtq = nc.dram_tensor("xtq", [D, SL], bf16, kind="ExternalInput").ap()
    wqt = nc.dram_tensor("wqt", [D, GD], bf16, kind="ExternalInput").ap()
    wkt = nc.dram_tensor("wkt", [D, GD], bf16, kind="ExternalInput").ap()
    wvt = nc.dram_tensor("wvt", [D, GD], bf16, kind="ExternalInput").ap()
    wota0 = nc.dram_tensor("wota0", [P, D], bf16, kind="ExternalInput").ap()
    wota1 = nc.dram_tensor("wota1", [65, D], bf16, kind="ExternalInput").ap()
    bias = nc.dram_tensor("bias", [P, 4], f32, kind="ExternalInput").ap()
    masks = nc.dram_tensor("masks", [P, 8 * QGS], bf16, kind="ExternalInput").ap()
    out = nc.dram_tensor("out", [SL, D], f32, kind="ExternalOutput").ap()

    Exp = mybir.ActivationFunctionType.Exp
    mult = mybir.AluOpType.mult
    add = mybir.AluOpType.add

    with tile.TileContext(nc) as tc, ExitStack() as ctx:
        const = ctx.enter_context(tc.tile_pool(name="const", bufs=1))

        # ---- persistent SBUF tiles ----
        xt_sb = const.tile([P, NC, S], bf16, tag="xt")
        xtq_sb = const.tile([P, NC, SL], bf16, tag="xtq")
        wqt_sb = const.tile([P, NC, GD], bf16, tag="wqt")
        wkt_sb = const.tile([P, NC, GD], bf16, tag="wkt")
        wvt_sb = const.tile([P, NC, GD], bf16, tag="wvt")
        wota0_sb = const.tile([P, D], bf16, tag="wota0")
        wota1_sb = const.tile([65, D], bf16, tag="wota1")
        bias_sb = const.tile([P, 4], f32, tag="bias")
        mask_sb = const.tile([P, 8 * QGS], bf16, tag="masks")
        kt01_sb = const.tile([P, S], bf16, tag="kt01")    # heads 0,1 stacked (64+64)
        kt2_sb = const.tile([64, S], bf16, tag="kt2")
        qt01_sb = const.tile([P, SL], bf16, tag="qt01")
        qt2_sb = const.tile([64, SL], bf16, tag="qt2")
        # V per head: [128 k-part, kb, 65] with col 64 = 1.0 (denominator)
        v_sb = [const.tile([P, NKB, 65], bf16, tag=f"v{h}", name=f"v{h}")
                for h in range(GH)]
        ones_sb = const.tile([1, 64], f32, tag="ones")

        # ---- load constants / inputs ----
        xt_r = xt.rearrange("(c p) s -> c p s", p=P)
        xtq_r = xtq.rearrange("(c p) s -> c p s", p=P)
        wqt_r = wqt.rearrange("(c p) d -> c p d", p=P)
        wkt_r = wkt.rearrange("(c p) d -> c p d", p=P)
        wvt_r = wvt.rearrange("(c p) d -> c p d", p=P)
        for c in range(NC):
            nc.sync.dma_start(wqt_sb[:, c, :], wqt_r[c])
            nc.sync.dma_start(xtq_sb[:, c, :], xtq_r[c])
            nc.sync.dma_start(wkt_sb[:, c, :], wkt_r[c])
            nc.sync.dma_start(wvt_sb[:, c, :], wvt_r[c])
            nc.sync.dma_start(xt_sb[:, c, :], xt_r[c])
        nc.sync.dma_start(wota0_sb[:], wota0[:])
        nc.sync.dma_start(wota1_sb[:], wota1[:])
        nc.sync.dma_start(bias_sb[:], bias[:])
        nc.sync.dma_start(mask_sb[:], masks[:])
        nc.vector.memset(ones_sb[:], 1.0)
        for h in range(GH):
            # whole tile -> 1.0; V evicts overwrite cols 0..63 of each block,
            # leaving col 64 as the denominator ones-column
            nc.vector.memset(v_sb[h][:], 1.0)

        # ---- interleaved projection chunks + attention groups ----
        # PSUM budget (8 banks): stA tag 2x[128,1024]=4, stB tag 1x[128,512]=1,
        # pv tag 3x[65,512]=3.  Projection psums + out-proj + bcast reuse the
        # same tags so attention group g can overlap projection chunk g+1.
        kq = [(kt01_sb, 0), (kt01_sb, 64), (kt2_sb, 0)]  # (tile, base partition)
        qq = [(qt01_sb, 0), (qt01_sb, 64), (qt2_sb, 0)]

        with tc.tile_pool(name="stA_ps", bufs=2, space="PSUM") as stAps, \
             tc.tile_pool(name="stB_ps", bufs=1, space="PSUM") as stBps, \
             tc.tile_pool(name="pv_ps", bufs=3, space="PSUM") as pvps, \
             tc.tile_pool(name="pt", bufs=5) as ptpool, \
             tc.tile_pool(name="stk", bufs=2) as stkpool, \
             tc.tile_pool(name="nrm", bufs=3) as nrmpool, \
             tc.tile_pool(name="oev", bufs=3) as oevpool:
            for qg in range(QG):
                # -- projection chunk qg: Q group qg, K blocks 8qg..8qg+7,
                #    V blocks 8qg..8qg+7 --
                ps = stAps.tile([P, QGS], f32, tag="stA", name=f"qps{qg}")
                for c in range(NC):
                    nc.tensor.matmul(
                        ps[:], wqt_sb[:, c, 0:128],
                        xtq_sb[:, c, qg * QGS:(qg + 1) * QGS],
                        start=(c == 0), stop=(c == NC - 1))
                nc.vector.tensor_scalar(
                    qt01_sb[:, qg * QGS:(qg + 1) * QGS], ps[:],
                    0.125, bias_sb[:, 0:1], mult, add)
                ps2 = stBps.tile([64, QGS], f32, tag="stB", name=f"qps2_{qg}")
                for c in range(NC):
                    nc.tensor.matmul(
                        ps2[:], wqt_sb[:, c, 128:192],
                        xtq_sb[:, c, qg * QGS:(qg + 1) * QGS],
                        start=(c == 0), stop=(c == NC - 1))
                nc.vector.tensor_scalar(
                    qt2_sb[:, qg * QGS:(qg + 1) * QGS], ps2[:],
                    0.125, bias_sb[0:64, 1:2], mult, add)
                for kg in (2 * qg, 2 * qg + 1):
                    ps = stAps.tile([P, QGS], f32, tag="stA", name=f"kps{kg}")
                    for c in range(NC):
                        nc.tensor.matmul(
                            ps[:], wkt_sb[:, c, 0:128],
                            xt_sb[:, c, kg * QGS:(kg + 1) * QGS],
                            start=(c == 0), stop=(c == NC - 1))
                    nc.vector.tensor_scalar(
                        kt01_sb[:, kg * QGS:(kg + 1) * QGS], ps[:],
                        bias_sb[:, 2:3], None, add)
                    ps2 = stBps.tile([64, QGS], f32, tag="stB", name=f"kps2_{kg}")
                    for c in range(NC):
                        nc.tensor.matmul(
                            ps2[:], wkt_sb[:, c, 128:192],
                            xt_sb[:, c, kg * QGS:(kg + 1) * QGS],
                            start=(c == 0), stop=(c == NC - 1))
                    nc.vector.tensor_scalar(
                        kt2_sb[:, kg * QGS:(kg + 1) * QGS], ps2[:],
                        bias_sb[0:64, 3:4], None, add)
                for kb in range(8 * qg, 8 * qg + 8):
                    psv = pvps.tile([P, GD], f32, tag="pv", name=f"vps{kb}")
                    for c in range(NC):
                        nc.tensor.matmul(
                            psv[:], xt_sb[:, c, kb * P:(kb + 1) * P], wvt_sb[:, c, :],
                            start=(c == 0), stop=(c == NC - 1))
                    for h in range(GH):
                        nc.vector.tensor_copy(
                            v_sb[h][:, kb, 0:64], psv[:, h * HD:(h + 1) * HD])

                # -- attention group qg --
                kcnt = 8 * (qg + 1)
                pv = [pvps.tile([65, QGS], f32, tag="pv", name=f"pv{qg}_{h}")
                      for h in range(GH)]
                for m in range(kcnt):
                    stA = stAps.tile([P, 2 * QGS], f32, tag="stA", name=f"stA{qg}_{m}")
                    stB = stBps.tile([P, QGS], f32, tag="stB", name=f"stB{qg}_{m}")
                    for h in range(GH):
                        kt_t, kb_p = kq[h]
                        qt_t, qb_p = qq[h]
                        dst = stA[:, h * QGS:(h + 1) * QGS] if h < 2 else stB[:]
                        nc.tensor.matmul(
                            dst,
                            kt_t[kb_p:kb_p + 64, m * P:(m + 1) * P],
                            qt_t[qb_p:qb_p + 64, qg * QGS:(qg + 1) * QGS],
                            start=True, stop=True)
                    ptA = ptpool.tile([P, 2 * QGS], bf16, tag="ptA")
                    ptB = ptpool.tile([P, QGS], bf16, tag="ptB")
                    nc.scalar.activation(ptA[:], stA[:], Exp)
                    nc.scalar.activation(ptB[:], stB[:], Exp)
                    if m >= 8 * qg:
                        # multiplicative 0/1 causal mask on the probabilities
                        w = m - 8 * qg
                        span = P * (w // 2 + 1)
                        for h in range(GH):
                            dst = (ptA[:, h * QGS:h * QGS + span] if h < 2
                                   else ptB[:, 0:span])
                            nc.vector.tensor_tensor(
                                dst, dst, mask_sb[:, w * QGS:w * QGS + span], mult)
                    for h in range(GH):
                        src = ptA[:, h * QGS:(h + 1) * QGS] if h < 2 else ptB[:]
                        nc.tensor.matmul(
                            pv[h][:], v_sb[h][:, m, :], src,
                            start=(m == 0), stop=(m == kcnt - 1))
                # normalize: out_h = pv_h[0:64] / pv_h[64] ; stack for out-proj
                stk0 = stkpool.tile([P, QGS], bf16, tag="sc0")
                stk1 = stkpool.tile([65, QGS], bf16, tag="sc1")
                nc.vector.memset(stk1[64:65, :], 1.0)
                for h in range(GH):
                    recip = nrmpool.tile([1, QGS], f32, tag="recip")
                    nc.vector.reciprocal(recip[:], pv[h][64:65, :])
                    bcast = stBps.tile([64, QGS], f32, tag="stB", name=f"bc{qg}_{h}")
                    nc.tensor.matmul(
                        bcast[:], ones_sb[:], recip[:], start=True, stop=True)
                    tmp = nrmpool.tile([64, QGS], bf16, tag="tmp")
                    nc.vector.tensor_copy(tmp[:], pv[h][0:64, :])
                    tgt = stk0[h * 64:(h + 1) * 64, :] if h < 2 else stk1[0:64, :]
                    nc.vector.tensor_tensor(tgt, tmp[:], bcast[:], mult)
                # output projection for this group's 4 query tiles
                for jj in range(QG):
                    op = stAps.tile([P, 1024], f32, tag="stA", name=f"op{qg}_{jj}")
                    for half in range(2):
                        nc.tensor.matmul(
                            op[:, half * 512:half * 512 + 384],
                            stk0[:, jj * P:(jj + 1) * P],
                            wota0_sb[:, half * 384:(half + 1) * 384],
                            start=True, stop=False)
                        nc.tensor.matmul(
                            op[:, half * 512:half * 512 + 384],
                            stk1[:, jj * P:(jj + 1) * P],
                            wota1_sb[:, half * 384:(half + 1) * 384],
                            start=False, stop=True)
                    oe = oevpool.tile([P, D], f32, tag="oe")
                    opv = op[:].rearrange("p (t x) -> p t x", x=512)
                    nc.vector.tensor_copy(
                        oe[:].rearrange("p (t x) -> p t x", x=384), opv[:, :, 0:384])
                    jq = 4 * qg + jj
                    nc.sync.dma_start(out[jq * P:(jq + 1) * P, :], oe[:])

    nc.compile()
    return nc


def _host_prep(inputs, Wq, bq, Wk, bk, Wv, bv, Wo, bo):
    import ml_dtypes

    bf16 = ml_dtypes.bfloat16
    X = np.asarray(inputs, np.float32).reshape(S, D)
    XT = np.ascontiguousarray(X.T)                      # [768, 4096]
    XT_bf = XT.astype(bf16)
    # query-set gathers: blocks s, s+2, ... of 32 128-col blocks
    XTb = XT.reshape(D, NKB // 2, 2, P)
    XTq = [np.ascontiguousarray(XTb[:, :, s, :].reshape(D, SL)).astype(bf16)
           for s in range(2)]

    # per-core multiplicative causal masks [128, 8*512], 1=keep 0=drop
    # (S_T layout: k on partitions, q on free dim)
    tri = (np.arange(P)[None, :] >= np.arange(P)[:, None]).astype(np.float32)
    mk = []
    for s_ in range(2):
        m = np.ones((P, 8, QGS), np.float32)
        for w in range(8):
            npref = max(0, -(-(w - s_) // 2))  # ceil((w - s)/2) clamped at 0
            m[:, w, :P * npref] = 0.0
            if w >= s_ and (w - s_) % 2 == 0:
                dblk = (w - s_) // 2
                m[:, w, dblk * P:(dblk + 1) * P] = tri
        mk.append(np.ascontiguousarray(m.reshape(P, 8 * QGS)).astype(bf16))

    in_maps = []
    for g in range(NG):
        hs = slice(GD * g, GD * (g + 1))
        WqT = np.ascontiguousarray(Wq[hs, :].T).astype(bf16)
        WkT = np.ascontiguousarray(Wk[hs, :].T).astype(bf16)
        WvT = np.ascontiguousarray(Wv[hs, :].T).astype(bf16)
        WoT = np.ascontiguousarray(Wo[:, hs].T).astype(np.float32)  # [192, 768]
        bo_g = bv[hs].astype(np.float32) @ WoT
        if g == 0:
            bo_g = bo_g + bo.astype(np.float32)
        wota = np.concatenate([WoT, bo_g[None, :]], axis=0)  # [193, 768]
        wota0 = np.ascontiguousarray(wota[0:P]).astype(bf16)
        wota1 = np.ascontiguousarray(wota[P:]).astype(bf16)
        bias_t = np.zeros((P, 4), np.float32)
        bias_t[:, 0] = bq[hs][0:128] / 8.0
        bias_t[0:64, 1] = bq[hs][128:192] / 8.0
        bias_t[:, 2] = bk[hs][0:128]
        bias_t[0:64, 3] = bk[hs][128:192]
        for s_ in range(2):
            in_maps.append({
                "xt": XT_bf, "xtq": XTq[s_],
                "wqt": WqT, "wkt": WkT, "wvt": WvT,
                "wota0": wota0, "wota1": wota1,
                "bias": bias_t, "masks": mk[s_],
            })
    return in_maps


def _gather(results):
    out = np.zeros((S, D), np.float32)
    ov = out.reshape(NQB, 2, P, D)
    for s_ in range(2):
        acc = np.zeros((SL, D), np.float32)
        for g in range(NG):
            acc += np.asarray(results[2 * g + s_]["out"], np.float32)
        ov[:, s_, :, :] = acc.reshape(NQB, P, D)
    return out.reshape(1, S, D)


def kernel(inputs, Wq, bq, Wk, bk, Wv, bv, Wo, bo):
    from concourse.bass_utils import run_bass_kernel_spmd

    if "nc" not in _CACHE:
        _CACHE["nc"] = _build_program()
    nc = _CACHE["nc"]
    in_maps = _host_prep(
        np.asarray(inputs), np.asarray(Wq), np.asarray(bq), np.asarray(Wk),
        np.asarray(bk), np.asarray(Wv), np.asarray(bv), np.asarray(Wo),
        np.asarray(bo))
    # core order: core = 2*g + s, but in_maps was built g-major with s inner,
    # i.e. in_maps[2*g + s] already matches core id 2*g + s.
    res = run_bass_kernel_spmd(nc, in_maps, list(range(8))).results
    return _gather(res)


# revision 32
# speedup vs baseline: 1.0638x; 1.0638x over previous
"""Multi-head causal self-attention (D=768, H=12, S=4096) on 8 Trainium2 cores.

Sharding: 4 head-groups (3 heads each) x 2 interleaved query-sets.
Core c = 2*g + s owns head-group g (heads 3g..3g+2) and query 128-row
blocks s, s+2, s+4, ... (even/odd interleave balances the causal
triangle).  Every core runs the SAME program; per-core behaviour is
driven entirely by input data (weight slices, gathered query columns,
additive causal masks).  Each core produces a partial [2048, 768]
output (its heads pushed through its slice of Wo, all biases folded
in); the host sums the 4 group partials per query-set and re-interleaves
rows.

Layout notes (per core):
  - scores are computed transposed: S_T[k, q] = K_h Q_h^T so the PV
    matmul needs no transposes; the softmax denominator comes from a
    ones-column appended to V.
  - softmax skips max-subtraction (scores are O(1) by construction).
  - causal masking is additive mask data (0 / -30000) applied to score
    PSUM with DVE before the exp, covering only the boundary k-blocks.
"""

import numpy as np

D = 768
S = 4096
H = 12
HD = 64
NG = 4          # head groups
GH = 3          # heads per group
GD = GH * HD    # 192 dims per group
SL = S // 2     # local queries per core (2048)
P = 128
NC = D // P     # 6 contraction chunks
QG = 4          # query groups per core (512 q each)
QGS = 512
NKB = S // P    # 32 key blocks
NQB = SL // P   # 16 local query tiles
MASKVAL = -30000.0

_CACHE = {}


def _build_program():
    import concourse.bacc as bacc
    import concourse.mybir as mybir
    import concourse.tile as tile
    from contextlib import ExitStack

    bf16 = mybir.dt.bfloat16
    f32 = mybir.dt.float32
    f32r = mybir.dt.float32r

    nc = bacc.Bacc("TRN2", target_bir_lowering=False, debug=False, num_devices=8)

    xt = nc.dram_tensor("xt", [D, S], bf16, kind="ExternalInput").ap()
    xtq = nc.dram_tensor("xtq", [D, SL], bf16, kind="ExternalInput").ap()
    wqt = nc.dram_tensor("wqt", [D, GD], bf16, kind="ExternalInput").ap()
    wkt = nc.dram_tensor("wkt", [D, GD], bf16, kind="ExternalInput").ap()
    wvt = nc.dram_tensor("wvt", [D, GD], bf16, kind="ExternalInput").ap()
    wota0 = nc.dram_tensor("wota0", [P, D], bf16, kind="ExternalInput").ap()
    wota1 = nc.dram_tensor("wota1", [65, D], bf16, kind="ExternalInput").ap()
    bias = nc.dram_tensor("bias", [P, 4], f32, kind="ExternalInput").ap()
    masks = nc.dram_tensor("masks", [P, 8 * QGS], bf16, kind="ExternalInput").ap()
    out = nc.dram_tensor("out", [SL, D], f32, kind="ExternalOutput").ap()

    Exp = mybir.ActivationFunctionType.Exp
    mult = mybir.AluOpType.mult
    add = mybir.AluOpType.add

    with tile.TileContext(nc) as tc, ExitStack() as ctx:
        const = ctx.enter_context(tc.tile_pool(name="const", bufs=1))

        # ---- persistent SBUF tiles ----
        xt_sb = const.tile([P, NC, S], bf16, tag="xt")
        xtq_sb = const.tile([P, NC, SL], bf16, tag="xtq")
        wqt_sb = const.tile([P, NC, GD], bf16, tag="wqt")
        wkt_sb = const.tile([P, NC, GD], bf16, tag="wkt")
        wvt_sb = const.tile([P, NC, GD], bf16, tag="wvt")
        wota0_sb = const.tile([P, D], bf16, tag="wota0")
        wota1_sb = const.tile([65, D], bf16, tag="wota1")
        bias_sb = const.tile([P, 4], f32, tag="bias")
        mask_sb = const.tile([P, 8 * QGS], bf16, tag="masks")
        kt01_sb = const.tile([P, S], bf16, tag="kt01")    # heads 0,1 stacked (64+64)
        kt2_sb = const.tile([64, S], bf16, tag="kt2")
        qt01_sb = const.tile([P, SL], bf16, tag="qt01")
        qt2_sb = const.tile([64, SL], bf16, tag="qt2")
        # V per head: [128 k-part, kb, 65] with col 64 = 1.0 (denominator)
        v_sb = [const.tile([P, NKB, 65], bf16, tag=f"v{h}", name=f"v{h}")
                for h in range(GH)]
        ones_sb = const.tile([1, 64], f32, tag="ones")

        # ---- load constants / inputs ----
        xt_r = xt.rearrange("(c p) s -> c p s", p=P)
        xtq_r = xtq.rearrange("(c p) s -> c p s", p=P)
        wqt_r = wqt.rearrange("(c p) d -> c p d", p=P)
        wkt_r = wkt.rearrange("(c p) d -> c p d", p=P)
        wvt_r = wvt.rearrange("(c p) d -> c p d", p=P)
        for c in range(NC):
            nc.sync.dma_start(wqt_sb[:, c, :], wqt_r[c])
            nc.sync.dma_start(xtq_sb[:, c, :], xtq_r[c])
            nc.sync.dma_start(wkt_sb[:, c, :], wkt_r[c])
            nc.sync.dma_start(wvt_sb[:, c, :], wvt_r[c])
            nc.sync.dma_start(xt_sb[:, c, :], xt_r[c])
        nc.sync.dma_start(wota0_sb[:], wota0[:])
        nc.sync.dma_start(wota1_sb[:], wota1[:])
        nc.sync.dma_start(bias_sb[:], bias[:])
        nc.sync.dma_start(mask_sb[:], masks[:])
        nc.vector.memset(ones_sb[:], 1.0)
        for h in range(GH):
            # whole tile -> 1.0; V evicts overwrite cols 0..63 of each block,
            # leaving col 64 as the denominator ones-column
            nc.vector.memset(v_sb[h][:], 1.0)

        # ---- interleaved projection chunks + attention groups ----
        # PSUM budget (8 banks): stA tag 2x[128,1024]=4, stB tag 1x[128,512]=1,
        # pv tag 3x[65,512]=3.  Projection psums + out-proj + bcast reuse the
        # same tags so attention group g can overlap projection chunk g+1.
        kq = [(kt01_sb, 0), (kt01_sb, 64), (kt2_sb, 0)]  # (tile, base partition)
        qq = [(qt01_sb, 0), (qt01_sb, 64), (qt2_sb, 0)]

        with tc.tile_pool(name="stA_ps", bufs=2, space="PSUM") as stAps, \
             tc.tile_pool(name="stB_ps", bufs=1, space="PSUM") as stBps, \
             tc.tile_pool(name="pv_ps", bufs=3, space="PSUM") as pvps, \
             tc.tile_pool(name="pt", bufs=5) as ptpool, \
             tc.tile_pool(name="stk", bufs=2) as stkpool, \
             tc.tile_pool(name="nrm", bufs=3) as nrmpool, \
             tc.tile_pool(name="oev", bufs=3) as oevpool:
            for qg in range(QG):
                # -- projection chunk qg: Q group qg, K blocks 8qg..8qg+7,
                #    V blocks 8qg..8qg+7 --
                ps = stAps.tile([P, QGS], f32, tag="stA", name=f"qps{qg}")
                for c in range(NC):
                    nc.tensor.matmul(
                        ps[:], wqt_sb[:, c, 0:128],
                        xtq_sb[:, c, qg * QGS:(qg + 1) * QGS],
                        start=(c == 0), stop=(c == NC - 1))
                nc.vector.tensor_scalar(
                    qt01_sb[:, qg * QGS:(qg + 1) * QGS], ps[:],
                    0.125, bias_sb[:, 0:1], mult, add)
                ps2 = stBps.tile([64, QGS], f32, tag="stB", name=f"qps2_{qg}")
                for c in range(NC):
                    nc.tensor.matmul(
                        ps2[:], wqt_sb[:, c, 128:192],
                        xtq_sb[:, c, qg * QGS:(qg + 1) * QGS],
                        start=(c == 0), stop=(c == NC - 1))
                nc.vector.tensor_scalar(
                    qt2_sb[:, qg * QGS:(qg + 1) * QGS], ps2[:],
                    0.125, bias_sb[0:64, 1:2], mult, add)
                for kg in (2 * qg, 2 * qg + 1):
                    ps = stAps.tile([P, QGS], f32, tag="stA", name=f"kps{kg}")
                    for c in range(NC):
                        nc.tensor.matmul(
                            ps[:], wkt_sb[:, c, 0:128],
                            xt_sb[:, c, kg * QGS:(kg + 1) * QGS],
                            start=(c == 0), stop=(c == NC - 1))
                    nc.vector.tensor_scalar(
                        kt01_sb[:, kg * QGS:(kg + 1) * QGS], ps[:],
                        bias_sb[:, 2:3], None, add)
                    ps2 = stBps.tile([64, QGS], f32, tag="stB", name=f"kps2_{kg}")
                    for c in range(NC):
                        nc.tensor.matmul(
                            ps2[:], wkt_sb[:, c, 128:192],
                            xt_sb[:, c, kg * QGS:(kg + 1) * QGS],
                            start=(c == 0), stop=(c == NC - 1))
                    nc.vector.tensor_scalar(
                        kt2_sb[:, kg * QGS:(kg + 1) * QGS], ps2[:],
                        bias_sb[0:64, 3:4], None, add)
                for kb in range(8 * qg, 8 * qg + 8):
                    psv = pvps.tile([P, GD], f32, tag="pv", name=f"vps{kb}")
                    for c in range(NC):
                        nc.tensor.matmul(
                            psv[:], xt_sb[:, c, kb * P:(kb + 1) * P], wvt_sb[:, c, :],
                            start=(c == 0), stop=(c == NC - 1))
                    for h in range(GH):
                        nc.vector.tensor_copy(
                            v_sb[h][:, kb, 0:64], psv[:, h * HD:(h + 1) * HD])

                # -- attention group qg --
                kcnt = 8 * (qg + 1)
                pv = [pvps.tile([65, QGS], f32, tag="pv", name=f"pv{qg}_{h}")
                      for h in range(GH)]
                for m in range(kcnt):
                    # causal query-suffix trim: for key block m, local query
                    # tiles j < ceil((m-s)/2) - 4qg are entirely below the
                    # diagonal for BOTH parities when using s=1's bound
                    # ceil((m-1)/2); parity-dependent leftovers are handled by
                    # the data mask inside the remaining span.
                    q0b = max(0, -(-(m - 1) // 2) - 4 * qg) if m > 0 else 0
                    q0 = P * q0b
                    vspan = QGS - q0
                    stA = stAps.tile([P, 2 * QGS], f32, tag="stA", name=f"stA{qg}_{m}")
                    stB = stBps.tile([P, QGS], f32, tag="stB", name=f"stB{qg}_{m}")
                    for h in range(GH):
                        kt_t, kb_p = kq[h]
                        qt_t, qb_p = qq[h]
                        dst = (stA[:, h * QGS + q0:(h + 1) * QGS] if h < 2
                               else stB[:, q0:])
                        nc.tensor.matmul(
                            dst,
                            kt_t[kb_p:kb_p + 64, m * P:(m + 1) * P],
                            qt_t[qb_p:qb_p + 64, qg * QGS + q0:(qg + 1) * QGS],
                            start=True, stop=True)
                    ptA = ptpool.tile([P, 2 * QGS], bf16, tag="ptA")
                    ptB = ptpool.tile([P, QGS], bf16, tag="ptB")
                    stA_v = stA[:].rearrange("p (h x) -> p h x", x=QGS)
                    ptA_v = ptA[:].rearrange("p (h x) -> p h x", x=QGS)
                    nc.scalar.activation(ptA_v[:, :, q0:], stA_v[:, :, q0:], Exp)
                    nc.scalar.activation(ptB[:, q0:], stB[:, q0:], Exp)
                    if m >= 8 * qg:
                        # multiplicative 0/1 causal mask on the probabilities
                        w = m - 8 * qg
                        span = P * (w // 2 + 1)
                        if span > q0:
                            for h in range(GH):
                                dst = (ptA[:, h * QGS + q0:h * QGS + span] if h < 2
                                       else ptB[:, q0:span])
                                nc.vector.tensor_tensor(
                                    dst, dst,
                                    mask_sb[:, w * QGS + q0:w * QGS + span], mult)
                    for h in range(GH):
                        src = (ptA[:, h * QGS + q0:(h + 1) * QGS] if h < 2
                               else ptB[:, q0:])
                        nc.tensor.matmul(
                            pv[h][:, q0:], v_sb[h][:, m, :], src,
                            start=(m == 0), stop=(m == kcnt - 1))
                # normalize: out_h = pv_h[0:64] / pv_h[64] ; stack for out-proj
                stk0 = stkpool.tile([P, QGS], bf16, tag="sc0")
                stk1 = stkpool.tile([65, QGS], bf16, tag="sc1")
                nc.vector.memset(stk1[64:65, :], 1.0)
                for h in range(GH):
                    recip = nrmpool.tile([1, QGS], f32, tag="recip")
                    nc.vector.reciprocal(recip[:], pv[h][64:65, :])
                    bcast = stBps.tile([64, QGS], f32, tag="stB", name=f"bc{qg}_{h}")
                    nc.tensor.matmul(
                        bcast[:], ones_sb[:], recip[:], start=True, stop=True)
                    tmp = nrmpool.tile([64, QGS], bf16, tag="tmp")
                    nc.vector.tensor_copy(tmp[:], pv[h][0:64, :])
                    tgt = stk0[h * 64:(h + 1) * 64, :] if h < 2 else stk1[0:64, :]
                    nc.vector.tensor_tensor(tgt, tmp[:], bcast[:], mult)
                # output projection for this group's 4 query tiles
                for jj in range(QG):
                    op = stAps.tile([P, 1024], f32, tag="stA", name=f"op{qg}_{jj}")
                    for half in range(2):
                        nc.tensor.matmul(
                            op[:, half * 512:half * 512 + 384],
                            stk0[:, jj * P:(jj + 1) * P],
                            wota0_sb[:, half * 384:(half + 1) * 384],
                            start=True, stop=False)
                        nc.tensor.matmul(
                            op[:, half * 512:half * 512 + 384],
                            stk1[:, jj * P:(jj + 1) * P],
                            wota1_sb[:, half * 384:(half + 1) * 384],
                            start=False, stop=True)
                    oe = oevpool.tile([P, D], f32, tag="oe")
                    opv = op[:].rearrange("p (t x) -> p t x", x=512)
                    nc.vector.tensor_copy(
                        oe[:].rearrange("p (t x) -> p t x", x=384), opv[:, :, 0:384])
                    jq = 4 * qg + jj
                    nc.sync.dma_start(out[jq * P:(jq + 1) * P, :], oe[:])

    nc.compile()
    return nc


def _host_prep(inputs, Wq, bq, Wk, bk, Wv, bv, Wo, bo):
    import ml_dtypes

    bf16 = ml_dtypes.bfloat16
    X = np.asarray(inputs, np.float32).reshape(S, D)
    XT = np.ascontiguousarray(X.T)                      # [768, 4096]
    XT_bf = XT.astype(bf16)
    # query-set gathers: blocks s, s+2, ... of 32 128-col blocks
    XTb = XT.reshape(D, NKB // 2, 2, P)
    XTq = [np.ascontiguousarray(XTb[:, :, s, :].reshape(D, SL)).astype(bf16)
           for s in range(2)]

    # per-core multiplicative causal masks [128, 8*512], 1=keep 0=drop
    # (S_T layout: k on partitions, q on free dim)
    tri = (np.arange(P)[None, :] >= np.arange(P)[:, None]).astype(np.float32)
    mk = []
    for s_ in range(2):
        m = np.ones((P, 8, QGS), np.float32)
        for w in range(8):
            npref = max(0, -(-(w - s_) // 2))  # ceil((w - s)/2) clamped at 0
            m[:, w, :P * npref] = 0.0
            if w >= s_ and (w - s_) % 2 == 0:
                dblk = (w - s_) // 2
                m[:, w, dblk * P:(dblk + 1) * P] = tri
        mk.append(np.ascontiguousarray(m.reshape(P, 8 * QGS)).astype(bf16))

    in_maps = []
    for g in range(NG):
        hs = slice(GD * g, GD * (g + 1))
        WqT = np.ascontiguousarray(Wq[hs, :].T).astype(bf16)
        WkT = np.ascontiguousarray(Wk[hs, :].T).astype(bf16)
        WvT = np.ascontiguousarray(Wv[hs, :].T).astype(bf16)
        WoT = np.ascontiguousarray(Wo[:, hs].T).astype(np.float32)  # [192, 768]
        bo_g = bv[hs].astype(np.float32) @ WoT
        if g == 0:
            bo_g = bo_g + bo.astype(np.float32)
        wota = np.concatenate([WoT, bo_g[None, :]], axis=0)  # [193, 768]
        wota0 = np.ascontiguousarray(wota[0:P]).astype(bf16)
        wota1 = np.ascontiguousarray(wota[P:]).astype(bf16)
        bias_t = np.zeros((P, 4), np.float32)
        bias_t[:, 0] = bq[hs][0:128] / 8.0
        bias_t[0:64, 1] = bq[hs][128:192] / 8.0
        bias_t[:, 2] = bk[hs][0:128]
        bias_t[0:64, 3] = bk[hs][128:192]
        for s_ in range(2):
            in_maps.append({
                "xt": XT_bf, "xtq": XTq[s_],
                "wqt": WqT, "wkt": WkT, "wvt": WvT,
                "wota0": wota0, "wota1": wota1,
                "bias": bias_t, "masks": mk[s_],
            })
    return in_maps


def _gather(results):
    out = np.zeros((S, D), np.float32)
    ov = out.reshape(NQB, 2, P, D)
    for s_ in range(2):
        acc = np.zeros((SL, D), np.float32)
        for g in range(NG):
            acc += np.asarray(results[2 * g + s_]["out"], np.float32)
        ov[:, s_, :, :] = acc.reshape(NQB, P, D)
    return out.reshape(1, S, D)


def kernel(inputs, Wq, bq, Wk, bk, Wv, bv, Wo, bo):
    from concourse.bass_utils import run_bass_kernel_spmd

    if "nc" not in _CACHE:
        _CACHE["nc"] = _build_program()
    nc = _CACHE["nc"]
    in_maps = _host_prep(
        np.asarray(inputs), np.asarray(Wq), np.asarray(bq), np.asarray(Wk),
        np.asarray(bk), np.asarray(Wv), np.asarray(bv), np.asarray(Wo),
        np.asarray(bo))
    # core order: core = 2*g + s, but in_maps was built g-major with s inner,
    # i.e. in_maps[2*g + s] already matches core id 2*g + s.
    res = run_bass_kernel_spmd(nc, in_maps, list(range(8))).results
    return _gather(res)
